# revision 20
# baseline (speedup 1.0000x reference)
"""Causal self-attention (B=2, S=2048, HID=1024, 16 heads x 64) on 8 trn2
NeuronCores.

Sharding: data-parallel over batch (cores 0-3 -> batch 0, cores 4-7 ->
batch 1), tensor-parallel over heads (4 heads per core via Wqk/Wv column
slices). Each core computes its 4 heads end-to-end; the [S, S] score
matrix stays core-local.

Per-core design (v3 — bf16, transpose-free, DMA-paced startup):
  - All matmul operands are bf16 (fp32 PSUM accumulation). Halves HBM
    traffic, enables FWL fast weight loads, avoids fp32r penalties.
  - q, k are produced TRANSPOSED ([head_cols, S]); scores are computed
    transposed ([sk, sq]) so exp(scores) feeds the P@V matmuls as lhsT.
  - P@V runs in [query-partition, head-dim-free] orientation:
    lhsT = pt[:, 128-query-block], rhs = v_aug [128, 65] (64 dims + a
    ones column that yields the softmax row-sum for free). Accumulated
    per query block in PSUM, then normalized in place with a
    per-partition reciprocal — no PE transposes, and the output DMA is
    in the natural [S, cols] layout.
  - Query blocks strictly left of the causal diagonal skip their P@V
    matmuls; the mask select covers only the 128-wide boundary block.
  - Emission is software-pipelined: the score matmul of unit i+1 is
    emitted before exp/PV of unit i, so the tensor engine streams
    score -> PV -> projection-filler work without stalling on the
    scalar engine's exp.
  - Inputs are loaded as wide-row tiles (wq|wk fused -> 1KB lines, x in
    [128, 1024] halves -> 2KB lines) to cut DMA packet overhead; the
    phase-1 projections interleave all K-chains chunk-by-chunk so the
    PE consumes each 128-row slab the moment its DMA lands.
  - A burst of dummy matmuls on memset tiles at t~0 (plus phase-1
    density) ramps the PE HAM clock gate to 2.4 GHz before the real
    work arrives; the exp activation table is likewise preloaded.
"""
import sys

for _p in ("/opt/trn_rl_repo",):
    if _p not in sys.path:
        sys.path.insert(0, _p)

import numpy as np

B, S, HID = 2, 2048, 1024
NH, HD = 16, 64
NHL = 4            # heads per core
WC = NHL * HD      # 256 local q/k weight cols
VC = NHL * (HD + 1)  # 260 local v cols incl. ones col
NT = S // 128      # 16 key chunks
NA = S // 512      # 4 query stripes
NK = HID // 128    # 8 contraction chunks

_NC = None


def _build(use_vbias=False):
    from concourse import bacc, mybir
    from concourse.tile import TileContext

    FP = mybir.dt.float32
    BF = mybir.dt.bfloat16
    Exp = mybir.ActivationFunctionType.Exp

    nc = bacc.Bacc("TRN2", target_bir_lowering=False, debug=False, num_devices=8)

    # all large inputs ride in one partition-major blob so the load is a
    # few big FIFO DMA pieces instead of ~35 fixed-cost-dominated ones.
    # per-partition col layout: 8 chunk-blocks [wqk|wv|x-qtr0] then x-qtr1
    # then x-half1
    CBW = 2 * WC + VC + 512          # 1284 cols per chunk block
    Q1O = NK * CBW                   # 10272: x qtr-1 section
    H1O = Q1O + NK * 512             # 14368: x half-1 section
    BLOBW = H1O + NK * 1024          # 22560
    blob_d = nc.dram_tensor("blob", [128, BLOBW], BF, kind="ExternalInput")
    bqk = nc.dram_tensor("bqk", [2 * WC, 1], FP, kind="ExternalInput")
    if use_vbias:
        wvl_d = nc.dram_tensor("wvl", [1, VC], BF, kind="ExternalInput")
        ones = nc.dram_tensor("ones", [1, 128], BF, kind="ExternalInput")
    out = nc.dram_tensor("out", [S, WC], FP, kind="ExternalOutput")

    with TileContext(nc) as tc:
        with (
            tc.tile_pool(name="inp", bufs=1) as inp,
            tc.tile_pool(name="ptp", bufs=3) as ptp,
            tc.tile_pool(name="outp", bufs=2) as outp,
            tc.tile_pool(name="rcp", bufs=8) as rcp,
            tc.tile_pool(name="Gs", bufs=2, space="PSUM") as gsp,
            tc.tile_pool(name="Gp", bufs=1, space="PSUM") as gpp,
            tc.tile_pool(name="O", bufs=3, space="PSUM") as opl,
        ):
            # ---- PE warm-up: dummy matmuls on memset tiles (no DMA deps)
            # keep the HAM clock gate busy so the real work runs at 2.4 GHz
            wz = inp.tile([128, 512], BF, name="wz")
            wo = inp.tile([128, 128], BF, name="wo")
            nc.vector.memset(wz[:, :], 0.0)
            nc.vector.memset(wo[:, :], 0.0)
            gw = gpp.tile([128, 512], mybir.dt.float32, tag="Gp", name="gw")
            for i in range(14):
                # full-array (K=128) so the HAM activity monitor actually
                # sees the PE as busy and lifts the clock gate
                nc.tensor.matmul(gw[:, :], lhsT=wo[:, :], rhs=wz[:, :],
                                 start=True, stop=True)
            if use_vbias:
                ones1 = inp.tile([1, 128], BF, name="ones1")
                nc.sync.dma_start(ones1[:, :], ones[:, :])
                wv_last = inp.tile([1, VC], BF, name="wvl")
                nc.sync.dma_start(wv_last[:, :], wvl_d[:, :])
            # preload the exp table set while the big DMAs stream in
            warm = inp.tile([1, 1], FP, name="warm")
            nc.vector.memset(warm[:, :], 0.0)
            nc.scalar.activation(warm[:, :], warm[:, :], Exp)

            # ---- persistent inputs in SBUF ----
            # 12 large DMA pieces: 8 chunk blocks (phase-1 prefix, ~330KB
            # each so the PE can chase the stream) go FIRST on the FIFO ring
            # -- each small transfer ahead of them would cost ~1us of fixed
            # completion latency before the first phase-1 slab could land
            cb = [inp.tile([128, CBW], BF, name=f"cb{k}") for k in range(NK)]
            for k in range(NK):
                nc.sync.dma_start(cb[k][:, :],
                                  blob_d[:, k * CBW:(k + 1) * CBW])
            bqk_sb = [inp.tile([128, 1], FP, name=f"bqk{t}") for t in range(4)]
            for t in range(4):
                nc.sync.dma_start(bqk_sb[t][:, :], bqk[t * 128:(t + 1) * 128, :])
            q1t = [inp.tile([128, 2048], BF, name=f"q1t{j}") for j in range(2)]
            for j in range(2):
                nc.sync.dma_start(
                    q1t[j][:, :],
                    blob_d[:, Q1O + j * 2048:Q1O + (j + 1) * 2048])
            h1t = [inp.tile([128, 4096], BF, name=f"h1t{j}") for j in range(2)]
            for j in range(2):
                nc.sync.dma_start(
                    h1t[j][:, :],
                    blob_d[:, H1O + j * 4096:H1O + (j + 1) * 4096])

            def wqk_view(k):
                return cb[k][:, 0:2 * WC]

            wv_k = [cb[k][:, 2 * WC:2 * WC + VC] for k in range(NK)]

            def xq(k, qtr):  # [128, 512] view of x quarter
                if qtr == 0:
                    return cb[k][:, 2 * WC + VC:CBW]
                if qtr == 1:
                    return q1t[k // 4][:, (k % 4) * 512:(k % 4) * 512 + 512]
                o = (k % 4) * 1024 + (qtr - 2) * 512
                return h1t[k // 4][:, o:o + 512]

            # split by S-quarter so interleaved later-quarter projection
            # writes can't false-depend against earlier attention reads
            qT_sb = [[inp.tile([128, 512], BF, name=f"qT{t}_{n}")
                      for n in range(4)] for t in range(2)]
            kT_sb = [[inp.tile([128, 512], BF, name=f"kT{t}_{n}")
                      for n in range(4)] for t in range(2)]

            def qview(ht, a, hb, c0, c1):
                return qT_sb[ht][a][hb:hb + 64, c0:c1]

            def kview(ht, kn, hb, c0, c1):
                return kT_sb[ht][kn][hb:hb + 64, c0:c1]
            v_sb = [inp.tile([128, VC], BF, name=f"v{c}") for c in range(NT)]

            # ---- projection emitters ----
            def proj_qk_unit(is_k, t, qtr):
                g = gpp.tile([128, 512], mybir.dt.float32, tag="Gp", name="g")
                co = (2 * WC // 2 if is_k else 0) + t * 128
                for k in range(NK):
                    nc.tensor.matmul(
                        g[:, :],
                        lhsT=wqk_view(k)[:, co:co + 128],
                        rhs=xq(k, qtr),
                        start=(k == 0), stop=(k == NK - 1),
                    )
                dst = kT_sb if is_k else qT_sb
                nc.vector.tensor_scalar_add(
                    dst[t][qtr][:, :], g[:, :], bqk_sb[2 * is_k + t][:, :]
                )

            def set_ones(c):
                # softmax row-sum column: constant 1.0 per head (the v-bias
                # row matmul is skipped -- biases are zeros per the spec)
                for h in range(NHL):
                    nc.vector.memset(v_sb[c][:, h * 65 + 64:h * 65 + 65], 1.0)

            def proj_v_chain(c, g):
                qtr, cc = divmod(c, 4)
                for k in range(NK):
                    nc.tensor.matmul(
                        g[:, :VC],
                        lhsT=xq(k, qtr)[:, cc * 128:(cc + 1) * 128],
                        rhs=wv_k[k][:, :],
                        start=(k == 0), stop=(not use_vbias and k == NK - 1),
                    )
                if use_vbias:
                    nc.tensor.matmul(  # bias row + ones column (K=1)
                        g[:, :VC], lhsT=ones1[:, :], rhs=wv_last[:, :],
                        start=False, stop=True,
                    )
                    nc.vector.tensor_copy(v_sb[c][:, :], g[:, :VC])
                else:
                    nc.vector.tensor_copy(v_sb[c][:, :], g[:, :VC])
                    set_ones(c)

            def proj_v_unit(c):
                g = gpp.tile([128, 512], mybir.dt.float32, tag="Gp", name="g")
                proj_v_chain(c, g)

            # ---- attention emitters (software-pipelined) ----
            # scores for one key chunk b of a head PAIR land in one G tile
            # ([h0-slice | h1-slice]); one exp covers both heads
            def emit_score(a, ht, b):
                g = gsp.tile([128, 1024], mybir.dt.float32, tag="Gs", name="g")
                kn, ko = divmod(b * 128, 512)
                off = max(0, (b - 4 * a) * 128)
                for hh in range(2):
                    hb = hh * 64
                    nc.tensor.matmul(
                        g[:, hh * 512 + off:(hh + 1) * 512],
                        lhsT=kview(ht, kn, hb, ko, ko + 128),
                        rhs=qview(ht, a, hb, off, 512),
                        start=True, stop=True,
                    )
                return g

            def emit_exp_pv(a, ht, b, nchunks, g, Os, onat):
                off = max(0, (b - 4 * a) * 128)
                j = off // 128  # first query block with any valid key here
                pt = ptp.tile([128, 1024], BF, tag="pt", name="pt")
                if off:
                    gv = g[:, :].rearrange("p (h w) -> p h w", h=2)[:, :, off:]
                    pv = pt[:, :].rearrange("p (h w) -> p h w", h=2)[:, :, off:]
                    nc.scalar.activation(pv, gv, Exp, scale=HD ** -0.5)
                else:
                    nc.scalar.activation(pt[:, :], g[:, :], Exp, scale=HD ** -0.5)
                if b >= 4 * a:
                    # triangular boundary block: zero entries above the
                    # diagonal (query j' < key p within the block)
                    for hh in range(2):
                        nc.gpsimd.affine_select(
                            out=pt[:, hh * 512 + off:hh * 512 + off + 128],
                            in_=pt[:, hh * 512 + off:hh * 512 + off + 128],
                            compare_op=mybir.AluOpType.is_ge,
                            fill=0.0, base=0,
                            pattern=[[1, 128]], channel_multiplier=-1,
                        )
                for hh in range(2):
                    h = 2 * ht + hh
                    for qb in range(4):
                        if qb < j:
                            continue  # query block entirely left of diagonal
                        # ONE start/stop per Os tile: start_tensor_calc marks
                        # the whole 2KB PSUM zero-region pending-zero, so a
                        # start per query-block group would wipe the other
                        # groups' accumulation state (overwrite on their next
                        # matmul). First write of each later group overwrites
                        # via the pending-zero left by the b=0/qb=0 start.
                        nc.tensor.matmul(
                            Os[hh][:, qb * 65:(qb + 1) * 65],
                            lhsT=pt[:, hh * 512 + qb * 128:hh * 512 + (qb + 1) * 128],
                            rhs=v_sb[b][:, h * 65:(h + 1) * 65],
                            start=(b == 0 and qb == j),
                            stop=(b == nchunks - 1 and qb == 3),
                            skip_group_check=True,
                        )
                # query block jq completes at chunk b = 4a+jq: normalize and
                # ship it immediately instead of bursting at the stripe end
                jq = b - 4 * a
                if 0 <= jq <= 3:
                    for hh in range(2):
                        h = 2 * ht + hh
                        recip = rcp.tile([128, 1], FP, tag="recip",
                                         name="recip")
                        nc.vector.reciprocal(
                            recip[:, :],
                            Os[hh][:, jq * 65 + 64:jq * 65 + 65])
                        nc.vector.tensor_scalar_mul(
                            onat[:, jq * WC + h * 64:jq * WC + (h + 1) * 64],
                            Os[hh][:, jq * 65:jq * 65 + 64],
                            recip[:, :])
                    if ht == 1 and jq % 2 == 1:
                        jh = jq // 2
                        dst = out[a * 512 + jh * 256:
                                  a * 512 + (jh + 1) * 256, :].rearrange(
                            "(qb p) c -> p qb c", qb=2)
                        nc.sync.dma_start(
                            dst, onat[:, jh * 512:(jh + 1) * 512].rearrange(
                                "p (qb c) -> p qb c", qb=2))

            # dummy keep-warm matmuls: fill PE idle slots (DMA waits in
            # phase 1, ACT-paced tail) so the HAM clock gate stays at 2.4GHz
            warm_g = {"t": None}

            def warm_fill():
                if warm_g["t"] is None:
                    warm_g["t"] = gpp.tile([128, 512], mybir.dt.float32,
                                           tag="Gp", name="gwt")
                nc.tensor.matmul(warm_g["t"][:, :], lhsT=wo[:, :],
                                 rhs=wz[:, :], start=True, stop=True)

            # ---- phase 1: all qtr-0 q/k projections plus v chunks 0-2,
            # K-chains interleaved per chunk so the PE tracks the DMA ----
            gq = gsp.tile([128, 1024], mybir.dt.float32, tag="Gs", name="gq")
            gq2 = gsp.tile([128, 1024], mybir.dt.float32, tag="Gs", name="gq2")
            vts = [opl.tile([128, VC], mybir.dt.float32, tag="O", name="vt")
                   for _ in range(3)]
            for k in range(NK):
                st, sp = (k == 0), (k == NK - 1)
                rhs = xq(k, 0)
                wk = wqk_view(k)
                nc.tensor.matmul(gq[:, 0:512], lhsT=wk[:, 0:128],
                                 rhs=rhs, start=st, stop=sp)
                nc.tensor.matmul(gq[:, 512:1024], lhsT=wk[:, 256:384],
                                 rhs=rhs, start=st, stop=sp)
                nc.tensor.matmul(gq2[:, 0:512], lhsT=wk[:, 128:256],
                                 rhs=rhs, start=st, stop=sp)
                nc.tensor.matmul(gq2[:, 512:1024], lhsT=wk[:, 384:512],
                                 rhs=rhs, start=st, stop=sp)
                for c in range(3):
                    nc.tensor.matmul(
                        vts[c][:, :VC],
                        lhsT=rhs[:, c * 128:(c + 1) * 128],
                        rhs=wv_k[k][:, :],
                        start=st, stop=(not use_vbias and sp),
                    )
            if use_vbias:
                for c in range(3):
                    nc.tensor.matmul(vts[c][:, :VC], lhsT=ones1[:, :],
                                     rhs=wv_last[:, :], start=False, stop=True)
            nc.vector.tensor_scalar_add(qT_sb[0][0][:, :], gq[:, 0:512],
                                        bqk_sb[0][:, :])
            nc.vector.tensor_scalar_add(kT_sb[0][0][:, :], gq[:, 512:1024],
                                        bqk_sb[2][:, :])
            nc.vector.tensor_scalar_add(qT_sb[1][0][:, :], gq2[:, 0:512],
                                        bqk_sb[1][:, :])
            nc.vector.tensor_scalar_add(kT_sb[1][0][:, :], gq2[:, 512:1024],
                                        bqk_sb[3][:, :])
            for c in range(3):
                nc.vector.tensor_copy(v_sb[c][:, :], vts[c][:, :VC])
                if not use_vbias:
                    set_ones(c)


            def q_(t, qtr):
                return lambda: proj_qk_unit(False, t, qtr)

            def k_(t, qtr):
                return lambda: proj_qk_unit(True, t, qtr)

            def v_(c):
                return lambda: proj_v_unit(c)

            filler = {
                0: [v_(3)],
                4: [q_(0, 1)], 5: [k_(0, 1)], 6: [v_(4)], 7: [v_(5)],
                8: [v_(6)], 9: [v_(7)], 10: [q_(1, 1)], 12: [k_(1, 1)],
                14: [q_(0, 2)], 17: [k_(0, 2)], 20: [q_(1, 2)], 23: [v_(8)],
                26: [v_(9)], 28: [v_(10)], 30: [v_(11)], 32: [k_(1, 2)],
                34: [q_(0, 3)], 36: [q_(1, 3)], 38: [k_(0, 3)],
                40: [k_(1, 3)], 42: [v_(12)], 44: [v_(14)], 46: [v_(13)],
                48: [v_(15)],
            }

            # ---- phases 2+3: pipelined attention ----
            units = []
            for a in range(NA):
                nchunks = 4 * a + 4
                for ht in range(2):
                    for b in range(nchunks):
                        units.append((a, ht, b, nchunks))


            os_by = {}       # (a, ht) -> [O tile hh0, O tile hh1]
            out_by_a = {}    # a -> 4 out_sb tiles [128, WC]
            pend = None      # (unit, g) emitted score awaiting exp+PV

            for i in range(len(units) + 1):
                if i < len(units):
                    a, ht, b, nchunks = units[i]
                    if (a, ht) not in os_by:
                        os_by[(a, ht)] = [
                            opl.tile([128, VC], mybir.dt.float32, tag="O",
                                     name="O") for _ in range(2)]
                    if a not in out_by_a:
                        out_by_a[a] = outp.tile(
                            [128, 4 * WC], FP, tag="onat", name="onat")
                    g = emit_score(a, ht, b)
                    nxt = (units[i], g)
                else:
                    nxt = None
                if pend is not None:
                    (a, ht, b, nchunks), g = pend
                    emit_exp_pv(a, ht, b, nchunks, g, os_by[(a, ht)],
                                out_by_a[a])
                    if b == nchunks - 1:
                        os_by.pop((a, ht))
                    u = i - 1
                    for f in filler.get(u, ()):
                        f()
                pend = nxt

    nc.compile()
    return nc


def _get_nc(use_vbias=False):
    global _NC
    if _NC is None or _NC[1] != use_vbias:
        _NC = (_build(use_vbias), use_vbias)
    return _NC[0]


def make_in_maps(hidden_states, Wqk, bqk, Wv, bv):
    import ml_dtypes

    bf16 = ml_dtypes.bfloat16
    x = np.asarray(hidden_states, dtype=np.float32)
    Wqk = np.asarray(Wqk, dtype=np.float32)
    bqk = np.asarray(bqk, dtype=np.float32)
    Wv = np.asarray(Wv, dtype=np.float32)
    bv = np.asarray(bv, dtype=np.float32)

    CBW = 2 * WC + VC + 512
    Q1O = 8 * CBW
    H1O = Q1O + 8 * 512
    BLOBW = H1O + 8 * 1024
    xTs = [np.ascontiguousarray(x[b].T.astype(bf16)) for b in range(B)]
    in_maps = []
    for c in range(8):
        b, ho = c // 4, (c % 4) * NHL
        cols = slice(ho * HD, (ho + NHL) * HD)
        wv_aug = np.zeros((HID + 1, VC), np.float32)
        for h in range(NHL):
            wv_aug[:HID, h * 65:h * 65 + HD] = Wv[:, (ho + h) * HD:(ho + h + 1) * HD]
            wv_aug[HID, h * 65:h * 65 + HD] = bv[(ho + h) * HD:(ho + h + 1) * HD]
            wv_aug[HID, h * 65 + HD] = 1.0
        wv_bf = wv_aug.astype(bf16)
        wqk_loc = np.concatenate(
            [Wqk[:, :HID][:, cols], Wqk[:, HID:][:, cols]], axis=1).astype(bf16)
        bqk_loc = np.concatenate(
            [bqk[:HID][cols], bqk[HID:][cols]]).reshape(2 * WC, 1)
        xT = xTs[b]
        blob = np.empty((128, BLOBW), bf16)
        for k in range(8):
            r = slice(k * 128, (k + 1) * 128)
            blob[:, k * CBW:k * CBW + 2 * WC] = wqk_loc[r, :]
            blob[:, k * CBW + 2 * WC:k * CBW + 2 * WC + VC] = wv_bf[r, :]
            blob[:, k * CBW + 2 * WC + VC:(k + 1) * CBW] = xT[r, 0:512]
            blob[:, Q1O + k * 512:Q1O + (k + 1) * 512] = xT[r, 512:1024]
            blob[:, H1O + k * 1024:H1O + (k + 1) * 1024] = xT[r, 1024:2048]
        m = {
            "blob": blob,
            "bqk": np.ascontiguousarray(bqk_loc),
        }
        if np.any(bv):
            m["wvl"] = np.ascontiguousarray(wv_bf[HID:HID + 1, :])
            m["ones"] = np.ones((1, 128), bf16)
        in_maps.append(m)
    return in_maps


def kernel(hidden_states, Wqk, bqk, Wv, bv):
    import time

    from concourse.bass_utils import run_bass_kernel_spmd

    in_maps = make_in_maps(hidden_states, Wqk, bqk, Wv, bv)
    use_vbias = bool(np.any(np.asarray(bv)))
    res = None
    for attempt in range(3):
        try:
            res = run_bass_kernel_spmd(_get_nc(use_vbias), in_maps,
                                       list(range(8)))
            break
        except Exception:
            # transient NRT_EXEC_UNIT_UNRECOVERABLE errors have been observed
            # on this fabric; back off and retry
            if attempt == 2:
                raise
            time.sleep(2.0)
    outp = np.empty((B, S, NH * HD), np.float32)
    for c in range(8):
        b, ho = c // 4, (c % 4) * NHL
        outp[b, :, ho * HD:(ho + NHL) * HD] = res.results[c]["out"]
    return outp
